# revision 26
# baseline (speedup 1.0000x reference)
"""BiLSTM Trainium2 kernel (V=128, H=512, B=512, S=256), 8 NeuronCores.

Sharding: 2 directions x 4 batch shards (128 batch rows per core).
Backward direction = forward scan on a time-reversed input sequence
(host reverses, so the device program is uniform SPMD).

Per-core algorithm (batch-major orientation, z-stationary):
  z_t = [onehot(x_t); h_{t-1}]  (K = V+H = 640, 5 K-tiles of 128)
  g_t[b, 4H] = z_t.T @ [WxT'; WhT]   (WxT' has bx+bh folded in, since
                                      sum_v onehot[v,b] == 1)
  i,f,o = sigmoid, gg = tanh  (gate-major columns, order i,f,o,g)
  c = f*c + i*gg ; h = o*tanh(c)
  hT (feature-major, 4 tiles of [128,128]) via TensorE transpose ->
  next step's stationary operand.
  FC (y_{t-1}[b,v] += h_{t-1}.T @ WfcHalf^T) rides the same stationary
  tiles one step behind; partial y summed across direction pairs on host.
"""

import numpy as np
import ml_dtypes

S, V, H, B = 256, 128, 512, 512
BC = 128  # batch per core
GH = 4 * H  # 2048
NCORES = 8

_BF16 = ml_dtypes.bfloat16

_cache = {}


def _build_nc(n_steps, n_exec=None):
    import concourse.bacc as bacc
    import concourse.tile as tile
    import concourse.mybir as mybir
    from concourse.masks import make_identity

    dt = mybir.dt
    AF = mybir.ActivationFunctionType

    if n_exec is None:
        n_exec = n_steps
    nc = bacc.Bacc("TRN2", target_bir_lowering=False, debug=False,
                   num_devices=NCORES)

    oh_d = nc.dram_tensor("oh", [n_steps, V, BC], dt.bfloat16, kind="ExternalInput")
    wt_d = nc.dram_tensor("wt", [5, 128, GH], dt.bfloat16, kind="ExternalInput")
    wfc_d = nc.dram_tensor("wfc", [4, 128, V], dt.bfloat16, kind="ExternalInput")
    y_d = nc.dram_tensor("y", [n_steps, BC, V], dt.float32, kind="ExternalOutput")

    # pass order within a step: (gate, half) with gates i(0), f(1), gg(3), o(2)
    # halves are 256-wide column groups of each gate's 512 columns.
    pass_list = [(0, 0), (1, 0), (3, 0), (2, 0), (0, 1), (1, 1), (3, 1), (2, 1)]

    with tile.TileContext(nc) as tc:
        with (
            tc.tile_pool(name="const", bufs=1) as const_pool,
            tc.tile_pool(name="oh", bufs=8) as oh_pool,
            tc.tile_pool(name="gsb", bufs=3) as gsb_pool,
            tc.tile_pool(name="tmp", bufs=4) as tmp_pool,
            tc.tile_pool(name="tau", bufs=3) as tau_pool,
            tc.tile_pool(name="cpool", bufs=1) as c_pool,
            tc.tile_pool(name="hbf", bufs=3) as h_pool,
            tc.tile_pool(name="hT", bufs=3) as hT_pool,
            tc.tile_pool(name="ysb", bufs=3) as y_pool,
            tc.tile_pool(name="gps", bufs=1, space="PSUM") as gps_pool,
            tc.tile_pool(name="hTps", bufs=2, space="PSUM") as hTps_pool,
            tc.tile_pool(name="yps", bufs=2, space="PSUM") as yps_pool,
        ):
            wt_sb = const_pool.tile([128, 5, GH], dt.bfloat16)
            nc.sync.dma_start(wt_sb[:], wt_d.rearrange("k p n -> p k n"))
            wfc_sb = const_pool.tile([128, 4, V], dt.bfloat16)
            nc.sync.dma_start(wfc_sb[:], wfc_d.rearrange("k p v -> p k v"))
            ident = const_pool.tile([128, 128], dt.bfloat16)
            make_identity(nc, ident[:])

            # Warm up the sigmoid/tanh ACT table set with dep-free ops so the
            # table-load pseudo-instruction doesn't land on a real gate
            # activation (walrus "too many sync wait commands" otherwise).
            warm = const_pool.tile([128, 16], dt.float32)
            nc.scalar.activation(warm[:], warm[:], AF.Sigmoid)
            nc.scalar.activation(warm[:], warm[:], AF.Tanh)

            c_t = c_pool.tile([128, H], dt.bfloat16)  # persistent cell state

            hT_prev = None  # [128, 4, 128] bf16, feature-major h of prev step
            y_ps_pending = None

            for t in range(n_exec):
                oh_t = oh_pool.tile([128, BC], dt.bfloat16)
                nc.sync.dma_start(oh_t[:], oh_d[t % n_steps])

                # one PSUM tile per gate (per bank): halves of the same gate
                # are 4 passes apart, so WAR deps between the two passes
                # sharing a bank never stall.
                g_ps = [gps_pool.tile([128, 512], dt.float32, tag=f"gps{g}",
                                      name=f"g_ps{g}_{t}")
                        for g in range(4)]
                g_sb = gsb_pool.tile([128, GH], dt.bfloat16)
                h_bf = h_pool.tile([128, H], dt.bfloat16)
                hT_ps = hTps_pool.tile([128, 4, 128], dt.bfloat16)
                hT_sb = hT_pool.tile([128, 4, 128], dt.bfloat16)

                for p_idx, (gate, half) in enumerate(pass_list):
                    wc = gate * H + half * 256
                    out_sl = g_ps[gate][:, half * 256:half * 256 + 256]
                    if t == 0:
                        nc.tensor.matmul(
                            out_sl, oh_t[:], wt_sb[:, 0, wc:wc + 256],
                            start=True, stop=True,
                        )
                    else:
                        for k in range(5):
                            lhsT = oh_t[:] if k == 0 else hT_prev[:, k - 1, :]
                            nc.tensor.matmul(
                                out_sl, lhsT, wt_sb[:, k, wc:wc + 256],
                                start=(k == 0), stop=(k == 4),
                            )
                            if p_idx == 0 and k >= 1:
                                nc.tensor.matmul(
                                    y_ps_pending, hT_prev[:, k - 1, :],
                                    wfc_sb[:, k - 1, :],
                                    start=(k == 1), stop=(k == 4),
                                )

                    func = AF.Tanh if gate == 3 else AF.Sigmoid
                    gc = gate * H + half * 256
                    nc.scalar.activation(g_sb[:, gc:gc + 256], out_sl, func)

                    if p_idx == 0 and t >= 1:
                        y_sb = y_pool.tile([128, V], dt.float32)
                        nc.vector.tensor_copy(y_sb[:], y_ps_pending[:])
                        nc.sync.dma_start(y_d[(t - 1) % n_steps], y_sb[:])
                        y_ps_pending = None

                    if (gate, half) == (2, 0) or (gate, half) == (2, 1):
                        hh = half
                        cs = slice(hh * 256, hh * 256 + 256)
                        sig_i = g_sb[:, 0 * H + hh * 256:0 * H + hh * 256 + 256]
                        sig_f = g_sb[:, 1 * H + hh * 256:1 * H + hh * 256 + 256]
                        sig_o = g_sb[:, 2 * H + hh * 256:2 * H + hh * 256 + 256]
                        tan_g = g_sb[:, 3 * H + hh * 256:3 * H + hh * 256 + 256]
                        if t == 0:
                            nc.vector.tensor_mul(c_t[:, cs], sig_i, tan_g)
                        else:
                            t2 = tmp_pool.tile([128, 256], dt.bfloat16, tag="t2")
                            nc.vector.tensor_mul(t2[:], sig_f, c_t[:, cs])
                            t1 = tmp_pool.tile([128, 256], dt.bfloat16, tag="t1")
                            nc.vector.tensor_mul(t1[:], sig_i, tan_g)
                            nc.vector.tensor_add(c_t[:, cs], t1[:], t2[:])
                        tau = tau_pool.tile([128, 256], dt.bfloat16)
                        nc.scalar.activation(tau[:], c_t[:, cs], AF.Tanh)
                        nc.vector.tensor_mul(h_bf[:, cs], sig_o, tau[:])
                        for j in (2 * hh, 2 * hh + 1):
                            nc.tensor.transpose(
                                hT_ps[:, j, :],
                                h_bf[:, j * 128:(j + 1) * 128],
                                ident[:],
                            )
                        nc.vector.tensor_copy(
                            hT_sb[:, 2 * hh:2 * hh + 2, :],
                            hT_ps[:, 2 * hh:2 * hh + 2, :],
                        )

                hT_prev = hT_sb
                if t + 1 < n_exec:
                    y_ps_pending = yps_pool.tile([128, V], dt.float32, tag="yps")

            # final FC for h_{S-1}
            y_ps = yps_pool.tile([128, V], dt.float32, tag="yps")
            for k in range(1, 5):
                nc.tensor.matmul(
                    y_ps[:], hT_prev[:, k - 1, :], wfc_sb[:, k - 1, :],
                    start=(k == 1), stop=(k == 4),
                )
            y_sb = y_pool.tile([128, V], dt.float32)
            nc.vector.tensor_copy(y_sb[:], y_ps[:])
            nc.sync.dma_start(y_d[(n_exec - 1) % n_steps], y_sb[:])

    nc.compile()
    return nc


def _get_nc(n_steps, n_exec=None):
    key = (n_steps, n_exec)
    if key not in _cache:
        _cache[key] = _build_nc(n_steps, n_exec)
    return _cache[key]


def _prep_core_inputs(x, Wx_f, Wh_f, bx_f, bh_f, Wx_b, Wh_b, bx_b, bh_b, Wfc,
                      n_steps):
    """Build the 8 per-core input maps. Cores 0-3: forward dir, shards 0-3.
    Cores 4-7: backward dir (time-reversed sequence), shards 0-3."""
    x = np.asarray(x)
    n_shards = B // BC
    eye = np.eye(V, dtype=_BF16)

    def wt_for(Wx, Wh, bx, bh):
        wxT = np.ascontiguousarray(np.transpose(np.asarray(Wx, np.float32),
                                                (2, 0, 1))).reshape(V, GH)
        bias = (np.asarray(bx, np.float32) + np.asarray(bh, np.float32)
                ).reshape(1, GH)
        whT = np.ascontiguousarray(np.transpose(np.asarray(Wh, np.float32),
                                                (2, 0, 1))).reshape(H, GH)
        wt = np.concatenate([wxT + bias, whT], axis=0)  # [640, 2048]
        return np.ascontiguousarray(wt.reshape(5, 128, GH).astype(_BF16))

    wt_f = wt_for(Wx_f, Wh_f, bx_f, bh_f)
    wt_b = wt_for(Wx_b, Wh_b, bx_b, bh_b)
    Wfc32 = np.asarray(Wfc, np.float32)
    wfc_f = np.ascontiguousarray(Wfc32[:, :H].T.reshape(4, 128, V).astype(_BF16))
    wfc_b = np.ascontiguousarray(Wfc32[:, H:].T.reshape(4, 128, V).astype(_BF16))

    in_maps = []
    for direction in range(2):
        for sh in range(n_shards):
            xs = x[sh * BC:(sh + 1) * BC, :n_steps]  # [BC, S]
            if direction == 1:
                xs = xs[:, ::-1]
            oh = eye[xs.T]  # [S, BC, V] one-hot
            oh = np.ascontiguousarray(np.transpose(oh, (0, 2, 1)))  # [S, V, BC]
            in_maps.append({
                "oh": oh,
                "wt": wt_f if direction == 0 else wt_b,
                "wfc": wfc_f if direction == 0 else wfc_b,
            })
    return in_maps


def _run(inputs, n_steps, trace=False):
    from concourse.bass_utils import run_bass_kernel_spmd

    nc = _get_nc(n_steps)
    in_maps = _prep_core_inputs(
        inputs["x"], inputs["Wx_f"], inputs["Wh_f"], inputs["bx_f"],
        inputs["bh_f"], inputs["Wx_b"], inputs["Wh_b"], inputs["bx_b"],
        inputs["bh_b"], inputs["Wfc"], n_steps)
    res = run_bass_kernel_spmd(nc, in_maps, list(range(NCORES)), trace=trace)

    bfc = np.asarray(inputs["bfc"], np.float32)
    n_shards = B // BC
    out = np.empty((B, n_steps, V), np.float32)
    for sh in range(n_shards):
        yf = res.results[sh]["y"]  # [S, BC, V]
        yb = res.results[n_shards + sh]["y"][::-1]  # flip time back
        y = yf + yb + bfc[None, None, :]
        out[sh * BC:(sh + 1) * BC] = np.transpose(y, (1, 0, 2))
    return out, res


def kernel(**inputs):
    out, _ = _run(inputs, S)
    return out
